# revision 12
# baseline (speedup 1.0000x reference)
"""Trainium2 Bass kernel for nn_ProbAttention (sparse attention / Informer ProbSparse).

Strategy (8 NeuronCores, no collectives):
  core c -> (batch b = c//2, half h = c%2).
  Both cores of a pair compute QK / M for their batch (collectives measured
  ~35us for a pair AllGather -- slower than the duplicated compute); the
  attention update (phase E) and the big Wfin product are split: each core
  only attends the selected queries that land in its 512-column shard.

Device pipeline per core (one batch, all-f16 PE path; selection verified
exact vs the fp32 reference on this dataset):
  B. K^T, Q^T (f16) from X^T/W f16; Q natural (own half), V(+ones col),
     vmean, Wadd residual.
  C. QK psum (f16 matmuls) tile-by-tile; fused DVE tensor_tensor_reduce
     (flag-mask multiply, max-reduce) -> maxacc; fused gpsimd
     scalar_tensor_tensor (qk/N * cnt, sum) -> sumacc. M = max - sum.
  D. rank[q] = #{j: M[j] > M[q]} for own-half queries only (DVE+gpsimd,
     vs a PE-broadcast M row); select rank < 140, compact own-half indices
     with gpsimd sparse_gather into 128 slots (pad sentinel 600 beyond
     nfound; own-half counts are 61..79 on this data, asserted < 128).
  E. One-hot gather Q_red (own-half sel only); scores^T = K @ Q_red^T; exp;
     attn@V with a ones-column in V giving denominators for free.
  F. Scatter aug rows + vmean fill into psum, add precomputed residual
     (+badd), 10 fused multiply-reduce dots against the prefetched Wfin
     shard split across DVE and gpsimd, partition-reduce by ones-matmul.

kernel(**inputs) is self-contained: host does layout prep only (permutation,
transposes, count masks from index_sample, Wfin reshape, f16 casts).
"""

import math
import sys

import numpy as np

sys.path.insert(0, "/opt/trn_rl_repo")

import concourse.bass as bass  # noqa: E402
import concourse.bacc as bacc  # noqa: E402
import concourse.tile as tile  # noqa: E402
from concourse import mybir  # noqa: E402
from concourse.bass_utils import run_bass_kernel_spmd  # noqa: E402

B, N, D, NCLS, U = 4, 1024, 256, 10, 140
F32 = mybir.dt.float32
F16 = mybir.dt.float16
ALU = mybir.AluOpType
ACTF = mybir.ActivationFunctionType
PAD = 600.0  # sentinel index for unused compact slots; matches no column


def build_nc(stage=9):
    nc = bacc.Bacc("TRN2", target_bir_lowering=False, debug=False, num_devices=8)

    xt_d = nc.declare_dram_parameter("xt_h", [D, N], F16, isOutput=False)
    w_d = nc.declare_dram_parameter("w_all_h", [8, 128, D], F16, isOutput=False)
    flag_d = nc.declare_dram_parameter("flag_b", [N, N], F16, isOutput=False)
    cnt_d = nc.declare_dram_parameter("cnt_b", [N, N], F16, isOutput=False)
    wf_d = nc.declare_dram_parameter("wfin_h", [NCLS, 128, N], F16, isOutput=False)
    misc_d = nc.declare_dram_parameter("misc", [128, 518], F32, isOutput=False)
    iwrap_d = nc.declare_dram_parameter("iwrap", [16, 64], F32, isOutput=False)
    out_d = nc.declare_dram_parameter("out10", [1, 16], F32, isOutput=True)

    mlin_d = nc.dram_tensor("m_lin", [N], F32)
    rklin_d = nc.dram_tensor("rank_lin", [512], F32)
    mtop_d = nc.dram_tensor("mtop_lin", [128], F32)

    def emit(tc):
        with (
            tc.tile_pool(name="const", bufs=1) as cpool,
            tc.tile_pool(name="wstream", bufs=1) as wpool,
            tc.tile_pool(name="big", bufs=1) as bpool,
            tc.tile_pool(name="maskA", bufs=3) as mpoolA,
            tc.tile_pool(name="maskB", bufs=3) as mpoolB,
            tc.tile_pool(name="scrA", bufs=2) as spoolA,
            tc.tile_pool(name="scrB", bufs=2) as spoolB,
            tc.tile_pool(name="small", bufs=1) as smpool,
        ):
            # ---- constant loads + memset consts ----
            xt = [cpool.tile([128, N], F16, name=f"xt{i}", tag=f"xt{i}") for i in range(2)]
            for ft in range(2):
                nc.sync.dma_start(xt[ft][:], xt_d[ft * 128:(ft + 1) * 128, :])
            wall = cpool.tile([128, 8 * D], F16, name="wall", tag="wall")
            nc.sync.dma_start(
                wall[:], bass.AP(w_d, 0, [[D, 128], [128 * D, 8], [1, D]])
            )
            wrb = {nm: [wall[:, (2 * i + ft) * D:(2 * i + ft + 1) * D] for ft in range(2)]
                   for i, nm in enumerate(("q", "k", "v", "a"))}
            misc = cpool.tile([128, 518], F32, name="misc", tag="misc")
            nc.sync.dma_start(misc[:], misc_d[:, :])
            nrow = misc[:, 0:512]
            qiota = misc[:, 512:516]
            badd = [misc[:, 516 + i:517 + i] for i in range(2)]
            iwrap = cpool.tile([16, 64], F32, name="iwrap", tag="iwrap")
            nc.sync.dma_start(iwrap[:], iwrap_d[:, :])
            onesr32 = cpool.tile([128, 1], F32, name="onesr32", tag="onesr32")
            nc.gpsimd.memset(onesr32[:], 1.0)
            onesrow32 = cpool.tile([1, 128], F32, name="onesrow32", tag="onesrow32")
            nc.gpsimd.memset(onesrow32[:], 1.0)
            ones16 = cpool.tile([128, 1], F16, name="ones16", tag="ones16")
            nc.gpsimd.memset(ones16[:], 1.0)
            # Wfin shard prefetch (used only at the very end)
            wf = [wpool.tile([128, N], F16, name=f"wf{c}", tag=f"wf{c}")
                  for c in range(NCLS)]
            for c in range(NCLS):
                nc.sync.dma_start(wf[c][:], wf_d[c, :, :])

            # ---- phase B1: K^T / Q^T in f16 ----
            ktT = [bpool.tile([128, N], F16, name=f"ktT{i}", tag=f"ktT{i}") for i in range(2)]
            qtT = [bpool.tile([128, N], F16, name=f"qtT{i}", tag=f"qtT{i}") for i in range(2)]
            qn = [bpool.tile([128, D], F16, name=f"qn{i}", tag=f"qn{i}") for i in range(4)]
            vnp = [bpool.tile([128, D + 1], F16, name=f"vnp{i}", tag=f"vnp{i}") for i in range(8)]
            for kt in range(8):
                nc.gpsimd.memset(vnp[kt][:, D:D + 1], 1.0)
            vmean_row = smpool.tile([1, D], F16, tag="vmean_row")
            resid16 = bpool.tile([128, 2 * 512], F16, name="resid16", tag="resid16")
            maxacc = smpool.tile([128, 8], F32, tag="maxacc")
            sumacc = smpool.tile([128, 8], F32, tag="sumacc")

            with tc.tile_pool(name="psA", bufs=2, space="PSUM") as psA:
                for wt, dst in ((wrb["k"], ktT), (wrb["q"], qtT)):
                    for et in range(2):
                        for nck in range(2):
                            ps = psA.tile([128, 512], F32, tag="psA")
                            for ft in range(2):
                                nc.tensor.matmul(
                                    ps[:],
                                    wt[ft][:, et * 128:(et + 1) * 128],
                                    xt[ft][:, nck * 512:(nck + 1) * 512],
                                    start=(ft == 0), stop=(ft == 1),
                                )
                            nc.scalar.copy(dst[et][:, nck * 512:(nck + 1) * 512], ps[:])

            # ---- phase C: QK + fused masked max / sampled sum ----
            with tc.tile_pool(name="psQK", bufs=2, space="PSUM") as psQK:
                for qt in range(8):
                    fg = mpoolA.tile([128, N], F16, tag="fg")
                    nc.sync.dma_start(fg[:], flag_d[qt * 128:(qt + 1) * 128, :])
                    ct2 = mpoolB.tile([128, N], F16, tag="ct2")
                    nc.sync.dma_start(ct2[:], cnt_d[qt * 128:(qt + 1) * 128, :])
                    qk = psQK.tile([128, N], F32, tag="qk")
                    for kc in range(2):
                        for et in range(2):
                            nc.tensor.matmul(
                                qk[:, kc * 512:(kc + 1) * 512],
                                qtT[et][:, qt * 128:(qt + 1) * 128],
                                ktT[et][:, kc * 512:(kc + 1) * 512],
                                start=(et == 0), stop=(et == 1),
                            )
                    # masked product qk*flag -> f16 SBUF; the max and the
                    # sampled sum (cnt=0 wherever flag=0, so product*cnt is
                    # exact) both read it back in 16-bit 2X mode.
                    # (tensor_tensor_reduce would fuse this but dies on HW.)
                    scrA = spoolA.tile([128, N], F16, tag="scrA")
                    nc.vector.tensor_mul(scrA[:], qk[:], fg[:])
                    nc.vector.tensor_reduce(
                        maxacc[:, qt:qt + 1], scrA[:], mybir.AxisListType.X,
                        ALU.max,
                    )
                    scrB = spoolB.tile([128, N], F16, tag="scrB")
                    nc.vector.scalar_tensor_tensor(
                        scrB[:], scrA[:], 1.0 / N, ct2[:], ALU.mult, ALU.mult,
                        accum_out=sumacc[:, qt:qt + 1],
                    )

            # ---- phase B2 (PE fills selection-latency gap) ----
            with tc.tile_pool(name="psB", bufs=2, space="PSUM") as psB:
                for qt in range(4):
                    ps = psB.tile([128, D], F32, tag="psb2")
                    for ft in range(2):
                        nc.tensor.matmul(
                            ps[:], xt[ft][:, qt * 128:(qt + 1) * 128],
                            wrb["q"][ft][:],
                            start=(ft == 0), stop=(ft == 1),
                        )
                    nc.scalar.copy(qn[qt][:], ps[:])
                for kt in range(8):
                    ps = psB.tile([128, D], F32, tag="psb2")
                    for ft in range(2):
                        nc.tensor.matmul(
                            ps[:], xt[ft][:, kt * 128:(kt + 1) * 128],
                            wrb["v"][ft][:],
                            start=(ft == 0), stop=(ft == 1),
                        )
                    nc.scalar.copy(vnp[kt][:, 0:D], ps[:])
                psvm = psB.tile([1, D], F32, tag="psvm", bufs=1)
                for kt in range(8):
                    nc.tensor.matmul(
                        psvm[:], ones16[:], vnp[kt][:, 0:D],
                        start=(kt == 0), stop=(kt == 7),
                    )
                nc.scalar.mul(vmean_row[:], psvm[:], 1.0 / N)
                for dtl in range(2):
                    ps = psB.tile([128, 512], F32, tag="psrd", bufs=2)
                    for ft in range(2):
                        nc.tensor.matmul(
                            ps[:], wrb["a"][ft][:, dtl * 128:(dtl + 1) * 128],
                            xt[ft][:, 0:512],
                            start=(ft == 0), stop=(ft == 1),
                        )
                    nc.scalar.activation(
                        resid16[:, dtl * 512:(dtl + 1) * 512], ps[:],
                        ACTF.Identity, bias=badd[dtl][:], scale=1.0,
                    )

            m_sb = smpool.tile([128, 8], F32, tag="m_sb")
            nc.vector.tensor_sub(m_sb[:], maxacc[:], sumacc[:])
            if stage == 1:
                nc.sync.dma_start(out_d[:, 0:8], m_sb[0:1, :])
                return

            # ---- phase D: rank-based top-140, own-half compact ----
            nc.sync.dma_start(bass.AP(mlin_d, 0, [[8, 128], [1, 8]]), m_sb[:])
            mrow_m = smpool.tile([1, N], F32, tag="mrow_m")
            nc.sync.dma_start(mrow_m[:], bass.AP(mlin_d, 0, [[N, 1], [1, N]]))
            rank = smpool.tile([128, 4], F32, tag="rank")
            mbc = bpool.tile([128, N], F32, name="mbc", tag="mbc")
            with tc.tile_pool(name="psM", bufs=1, space="PSUM") as psM:
                psm = psM.tile([128, N], F32, tag="psm")
                for hh in range(2):
                    nc.tensor.matmul(
                        psm[:, hh * 512:(hh + 1) * 512], onesrow32[:],
                        mrow_m[:, hh * 512:(hh + 1) * 512],
                        start=True, stop=True,
                    )
                nc.scalar.copy(mbc[:], psm[:])
                for qt in range(4):
                    scr = (spoolA if qt < 2 else spoolB).tile([128, N], F16, tag="scrR")
                    nc.vector.tensor_scalar(
                        scr[:], mbc[:], m_sb[:, qt:qt + 1], None, ALU.is_gt,
                        ALU.add, accum_out=rank[:, qt:qt + 1],
                    )
            if stage == 2:
                nc.sync.dma_start(out_d[:, 0:4], rank[0:1, :])
                return
            nc.sync.dma_start(bass.AP(rklin_d, 0, [[4, 128], [1, 4]]), rank[:])
            rkw = smpool.tile([16, 64], F32, tag="rkw")
            nc.gpsimd.memset(rkw[:, 32:64], 0.0)
            nc.sync.dma_start(rkw[:, 0:32], bass.AP(rklin_d, 0, [[32, 16], [1, 32]]))
            sel1 = smpool.tile([16, 64], F32, tag="sel1")
            nc.vector.tensor_scalar(sel1[:], rkw[:], 139.5, None, ALU.is_le)
            selv = smpool.tile([16, 64], F32, tag="selv")
            nc.vector.scalar_tensor_tensor(
                selv[:], sel1[:], 1.0, iwrap[:], ALU.mult, ALU.mult
            )
            selv3 = smpool.tile([16, 64], F32, tag="selv3")
            nc.vector.tensor_scalar(selv3[:], selv[:], -1.0, None, ALU.add)
            compact = smpool.tile([16, 16], F32, tag="compact")
            nfound = smpool.tile([1, 1], mybir.dt.uint32, tag="nfound")
            nc.gpsimd.sparse_gather(compact[:], selv3[:], num_found=nfound[:])
            nc.scalar.dma_start(
                bass.AP(mtop_d, 0, [[1, 16], [16, 8]]), compact[:, 0:8]
            )
            mtc0 = smpool.tile([128, 1], F32, tag="mtc0")
            nc.scalar.dma_start(mtc0[:], bass.AP(mtop_d, 0, [[1, 128], [1, 1]]))
            mrow1 = smpool.tile([1, 128], F32, tag="mrow1")
            nc.scalar.dma_start(mrow1[:], bass.AP(mtop_d, 0, [[128, 1], [1, 128]]))
            if stage == 3:
                nc.sync.dma_start(out_d[:, 0:8], compact[0:1, :])
                nc.sync.dma_start(out_d[:, 8:9], mtc0[0:1, :])
                return

            # ---- phase E: Q_red gather, scores^T, exp, attn@V (own half) ----
            e2 = [smpool.tile([128, 128], F16, name=f"e2_{i}", tag=f"e2_{i}")
                  for i in range(4)]
            er0 = smpool.tile([128, 512], F16, tag="er0")
            qredT = [smpool.tile([128, 128], F16, name=f"qredT{i}", tag=f"qredT{i}")
                     for i in range(2)]
            expdT = [smpool.tile([128, 128], F16, name=f"expdT{i}", tag=f"expdT{i}")
                     for i in range(8)]
            aug0 = smpool.tile([128, D], F16, tag="aug0")
            ctxh = bpool.tile([128, 2 * 512], F16, name="ctxh", tag="ctxh")
            facc = smpool.tile([128, 16], F32, tag="facc")
            nc.gpsimd.memset(facc[:, NCLS:16], 0.0)
            with tc.tile_pool(name="psR", bufs=1, space="PSUM") as psR, \
                 tc.tile_pool(name="psC", bufs=2, space="PSUM") as psC, \
                 tc.tile_pool(name="psE", bufs=1, space="PSUM") as psE:
                psr = psR.tile([128, 128], F32, tag="psr")
                nc.tensor.matmul(psr[:], onesrow32[:], mrow1[:], start=True, stop=True)
                for qt in range(4):
                    nc.vector.tensor_scalar(
                        e2[qt][:], psr[:], qiota[:, qt:qt + 1], None, ALU.is_equal
                    )
                nc.vector.tensor_scalar(er0[:], nrow[:], mtc0[:], None, ALU.is_equal)
                for ec in range(2):
                    ps = psC.tile([128, 128], F32, tag="psC")
                    for qt in range(4):
                        nc.tensor.matmul(
                            ps[:], qn[qt][:, ec * 128:(ec + 1) * 128], e2[qt][:],
                            start=(qt == 0), stop=(qt == 3),
                        )
                    nc.scalar.copy(qredT[ec][:], ps[:])
                for kt in range(8):
                    ps = psC.tile([128, 128], F32, tag="psC")
                    for et in range(2):
                        nc.tensor.matmul(
                            ps[:], ktT[et][:, kt * 128:(kt + 1) * 128], qredT[et][:],
                            start=(et == 0), stop=(et == 1),
                        )
                    nc.scalar.activation(
                        expdT[kt][:], ps[:], ACTF.Exp, scale=1.0 / math.sqrt(D)
                    )
                pse = psE.tile([128, D + 1], F32, tag="pse")
                for kt in range(8):
                    nc.tensor.matmul(
                        pse[:], expdT[kt][:], vnp[kt][:],
                        start=(kt == 0), stop=(kt == 7),
                    )
                rc = smpool.tile([128, 1], F32, tag="rc")
                nc.vector.reciprocal(rc[:], pse[:, D:D + 1])
                nc.vector.tensor_scalar(aug0[:], pse[:, 0:D], rc[:], None, ALU.mult)
            if stage == 4:
                nc.sync.dma_start(out_d[:, :], aug0[0:1, 0:16])
                return

            # ---- phase F: scatter + fill + residual add + fused dots ----
            with tc.tile_pool(name="psF", bufs=2, space="PSUM") as psF, \
                 tc.tile_pool(name="psCt", bufs=1, space="PSUM") as psCt:
                cnt_ps = psCt.tile([1, 512], F32, tag="cnt_ps")
                nc.tensor.matmul(cnt_ps[:], ones16[:], er0[:], start=True, stop=True)
                fill16 = smpool.tile([1, 512], F16, tag="fill16")
                nc.scalar.activation(fill16[:], cnt_ps[:], ACTF.Copy,
                                     bias=1.0, scale=-1.0)
                for dtl in range(2):
                    ps = psF.tile([128, 512], F32, tag="psF")
                    nc.tensor.matmul(
                        ps[:], aug0[:, dtl * 128:(dtl + 1) * 128], er0[:],
                        start=True, stop=False,
                    )
                    nc.tensor.matmul(
                        ps[:], vmean_row[0:1, dtl * 128:(dtl + 1) * 128], fill16[:],
                        start=False, stop=True,
                    )
                    nc.vector.scalar_tensor_tensor(
                        ctxh[:, dtl * 512:(dtl + 1) * 512], ps[:], 1.0,
                        resid16[:, dtl * 512:(dtl + 1) * 512], ALU.mult, ALU.add,
                    )
                if stage == 5:
                    nc.sync.dma_start(out_d[:, :], ctxh[0:1, 0:16].bitcast(F16))
                    return
                for cls in range(NCLS):
                    scr = (spoolA if cls % 2 else spoolB).tile([128, N], F16, tag="scrD")
                    nc.vector.scalar_tensor_tensor(
                        scr[:], ctxh[:], 1.0, wf[cls][:], ALU.mult, ALU.mult,
                        accum_out=facc[:, cls:cls + 1],
                    )
            with tc.tile_pool(name="psO", bufs=1, space="PSUM") as psO:
                o = psO.tile([1, 16], F32, tag="o")
                nc.tensor.matmul(o[:], onesr32[:], facc[:], start=True, stop=True)
                osb = smpool.tile([1, 16], F32, tag="osb")
                nc.scalar.copy(osb[:], o[:])
                nc.sync.dma_start(out_d[:, :], osb[:])

    with tile.TileContext(nc) as tc:
        emit(tc)
    nc.compile()
    return nc


_NC_CACHE = {}


def get_nc(stage=9):
    if stage not in _NC_CACHE:
        _NC_CACHE[stage] = build_nc(stage)
    return _NC_CACHE[stage]


def host_prep(inputs):
    """Build per-core input maps from the full problem inputs (layout only)."""
    x = np.asarray(inputs["input_embedding"], np.float32)        # [B, N, D]
    wq = np.asarray(inputs["Wq"], np.float32)
    wk = np.asarray(inputs["Wk"], np.float32)
    wv = np.asarray(inputs["Wv"], np.float32)
    wa = np.asarray(inputs["Wadd"], np.float32)
    badd = np.asarray(inputs["badd"], np.float32)
    wfin = np.asarray(inputs["Wfin"], np.float32)                # [10, N*D]
    idx = np.asarray(inputs["index_sample"]).astype(np.int64)    # [N, U]

    cnt = np.zeros((N, N), np.float32)
    np.add.at(cnt, (np.arange(N)[:, None], idx), 1.0)

    # Core half h=1 gets the n-axis halves swapped on every n-indexed input
    # (the pipeline is equivariant under a joint permutation of X rows,
    # mask rows+cols, and Wfin columns), so "columns 0:512" is its half.
    perms = [np.arange(N), np.concatenate([np.arange(512, N), np.arange(512)])]
    flag_h, cnt_h = [], []
    for p in perms:
        cp = cnt[p][:, p]
        flag_h.append((cp > 0).astype(np.float16))
        cnt_h.append(cp.astype(np.float16))

    # Wfin[c, n*256+d] -> [10, d, n_local] -> [10, 128, 2*512] f16
    wr = wfin.reshape(NCLS, N, D).transpose(0, 2, 1)             # [10, 256, 1024]
    wr_h = [
        np.ascontiguousarray(
            wr[:, :, perms[h][:512]].reshape(NCLS, 2, 128, 512)
            .transpose(0, 2, 1, 3)
        ).reshape(NCLS, 128, N).astype(np.float16)
        for h in range(2)
    ]

    w_all = np.stack([w.T.reshape(2, 128, D) for w in (wq, wk, wv, wa)])
    misc = np.zeros((128, 518), np.float32)
    misc[:, 0:512] = np.arange(512, dtype=np.float32)[None, :]
    misc[:, 512:516] = (np.arange(128, dtype=np.float32)[:, None]
                        + 128.0 * np.arange(4, dtype=np.float32)[None, :])
    misc[:, 516] = badd[0:128]
    misc[:, 517] = badd[128:256]
    _s = (np.arange(16, dtype=np.float32)[:, None] * 32
          + np.arange(32, dtype=np.float32)[None, :])
    # cols 0:32 -> own-half index+1 by rank; 32:40 -> 128 PAD sentinels
    # (keeps sparse_gather finds <= 256); 40:64 -> 0 (never found).
    iwrap = np.zeros((16, 64), np.float32)
    iwrap[:, 0:32] = 1.0 + (_s % 4) * 128 + _s // 4
    iwrap[:, 32:40] = PAD + 1.0
    consts = {
        "w_all_h": np.ascontiguousarray(w_all.reshape(8, 128, D)).astype(np.float16),
        "misc": misc,
        "iwrap": iwrap,
    }

    in_maps = []
    xt_cache = {}
    for c in range(8):
        b, h = c // 2, c % 2
        m = dict(consts)
        if (b, h) not in xt_cache:
            xp = np.ascontiguousarray(x[b][perms[h]])
            xt_cache[(b, h)] = np.ascontiguousarray(xp.T).astype(np.float16)
        m["xt_h"] = xt_cache[(b, h)]
        m["flag_b"] = flag_h[h]
        m["cnt_b"] = cnt_h[h]
        m["wfin_h"] = wr_h[h]
        in_maps.append(m)
    return in_maps


def host_combine(results, inputs):
    bfin = np.asarray(inputs["bfin"], np.float32)
    out = np.zeros((B, NCLS), np.float32)
    for c in range(8):
        b = c // 2
        out[b] += results[c]["out10"].reshape(-1)[0:NCLS]
    return out + bfin[None, :]


def kernel(**inputs):
    nc = get_nc()
    in_maps = host_prep(inputs)
    res = run_bass_kernel_spmd(nc, in_maps, core_ids=list(range(8)))
    return host_combine(res.results, inputs)


# revision 17
# speedup vs baseline: 1.2237x; 1.2237x over previous
"""Trainium2 Bass kernel for nn_ProbAttention (sparse attention / Informer ProbSparse).

Strategy (8 NeuronCores, no collectives):
  core c -> (batch b = c//2, half h = c%2).
  Both cores of a pair compute QK / M for their batch (a pair AllGather
  measured ~35us -- slower than the duplicated compute); the attention
  update and the big Wfin product are column-split: each core only attends
  the selected queries that land in its 512-column shard.

Device pipeline per core (one batch, bf16 PE path; max 2 top-140 selection
swaps vs the fp32 reference on this dataset, rel err ~3e-3 << 2e-2):
  B. K^T, Q^T (bf16) from X^T/W bf16; V(+ones col), vmean, Wadd residual.
  C. QK into PSUM (bf16 matmuls) + additive -30000 sample mask accumulated
     on the PE (ident @ am); DVE reduce-max -> maxacc and fused
     scalar_tensor_tensor (qk/N * cnt, sum) -> sumacc. M = max - sum.
  D. No index compaction at all: M row broadcast via PE (transpose +
     ones-row matmuls), rank[q] = #{j: M[j] > M[q]} for own-half queries
     (4 DVE ops), selm = rank < 140, and the scatter one-hots
     D[q, col] = (col == q) * selm[q] built by one fused tensor_scalar per
     128-query chunk. No DRAM roundtrips, no gpsimd.
  E. scores^T = K^T-slices @ Q^T(own half) for ALL 512 own queries; exp on
     ACT; attn@V with a ones-column in V giving denominators for free.
  F. Scatter aug rows + vmean fill into PSUM via D, add precomputed
     residual (+badd), 10 fused multiply-reduce dots against the
     prefetched Wfin shard, partition-reduce by ones-matmul.

kernel(**inputs) is self-contained: host does layout prep only (permutation,
transposes, count masks from index_sample, Wfin reshape, bf16 casts).
"""

import math
import sys

import numpy as np

sys.path.insert(0, "/opt/trn_rl_repo")

import concourse.bass as bass  # noqa: E402
import concourse.bacc as bacc  # noqa: E402
import concourse.tile as tile  # noqa: E402
from concourse import mybir  # noqa: E402
from concourse.bass_utils import run_bass_kernel_spmd  # noqa: E402

import ml_dtypes  # noqa: E402

B, N, D, NCLS, U = 4, 1024, 256, 10, 140
F32 = mybir.dt.float32
BF16 = mybir.dt.bfloat16
ALU = mybir.AluOpType
ACTF = mybir.ActivationFunctionType
NEG = -30000.0


def build_nc(stage=9):
    nc = bacc.Bacc("TRN2", target_bir_lowering=False, debug=False, num_devices=8)

    xt_d = nc.declare_dram_parameter("xt_h", [D, N], BF16, isOutput=False)
    w_d = nc.declare_dram_parameter("w_all_h", [8, 128, D], BF16, isOutput=False)
    am_d = nc.declare_dram_parameter("am_b", [N, N], BF16, isOutput=False)
    cnt_d = nc.declare_dram_parameter("cnt_b", [N, N], BF16, isOutput=False)
    wf_d = nc.declare_dram_parameter("wfin_h", [NCLS, 128, N], BF16, isOutput=False)
    misc_d = nc.declare_dram_parameter("misc", [128, 518], F32, isOutput=False)
    id32_d = nc.declare_dram_parameter("ident32", [128, 128], F32, isOutput=False)
    idb_d = nc.declare_dram_parameter("identb", [128, 128], BF16, isOutput=False)
    sel8_d = nc.declare_dram_parameter("sel8", [8, 1024], F32, isOutput=False)
    out_d = nc.declare_dram_parameter("out10", [1, 16], F32, isOutput=True)

    def emit(tc):
        with (
            tc.tile_pool(name="const", bufs=1) as cpool,
            tc.tile_pool(name="wstream", bufs=1) as wpool,
            tc.tile_pool(name="big", bufs=1) as bpool,
            tc.tile_pool(name="maskA", bufs=3) as mpoolA,
            tc.tile_pool(name="maskB", bufs=3) as mpoolB,
            tc.tile_pool(name="scrA", bufs=2) as spoolA,
            tc.tile_pool(name="scrB", bufs=2) as spoolB,
            tc.tile_pool(name="small", bufs=1) as smpool,
        ):
            # ---- constant loads + memset consts ----
            xt = [cpool.tile([128, N], BF16, name=f"xt{i}", tag=f"xt{i}") for i in range(2)]
            for ft in range(2):
                nc.sync.dma_start(xt[ft][:], xt_d[ft * 128:(ft + 1) * 128, :])
            wall = cpool.tile([128, 8 * D], BF16, name="wall", tag="wall")
            nc.sync.dma_start(
                wall[:], bass.AP(w_d, 0, [[D, 128], [128 * D, 8], [1, D]])
            )
            wrb = {nm: [wall[:, (2 * i + ft) * D:(2 * i + ft + 1) * D] for ft in range(2)]
                   for i, nm in enumerate(("q", "k", "v", "a"))}
            misc = cpool.tile([128, 518], F32, name="misc", tag="misc")
            nc.sync.dma_start(misc[:], misc_d[:, :])
            nrow = misc[:, 0:512]
            qiota = misc[:, 512:516]
            badd = [misc[:, 516 + i:517 + i] for i in range(2)]
            ident32 = cpool.tile([128, 128], F32, name="ident32", tag="ident32")
            nc.sync.dma_start(ident32[:], id32_d[:, :])
            identb = cpool.tile([128, 128], BF16, name="identb", tag="identb")
            nc.sync.dma_start(identb[:], idb_d[:, :])
            sel8 = cpool.tile([8, 1024], F32, name="sel8", tag="sel8")
            nc.sync.dma_start(sel8[:], sel8_d[:, :])
            onesrow32 = cpool.tile([1, 128], F32, name="onesrow32", tag="onesrow32")
            nc.gpsimd.memset(onesrow32[:], 1.0)
            ones16 = cpool.tile([128, 1], BF16, name="ones16", tag="ones16")
            nc.gpsimd.memset(ones16[:], 1.0)
            onesr32 = cpool.tile([128, 1], F32, name="onesr32", tag="onesr32")
            nc.gpsimd.memset(onesr32[:], 1.0)
            wf = [wpool.tile([128, N], BF16, name=f"wf{c}", tag=f"wf{c}")
                  for c in range(NCLS)]
            for c in range(NCLS):
                nc.sync.dma_start(wf[c][:], wf_d[c, :, :])

            # ---- phase B1: K^T / Q^T in bf16 ----
            ktT = [bpool.tile([128, N], BF16, name=f"ktT{i}", tag=f"ktT{i}") for i in range(2)]
            qtT = [bpool.tile([128, N], BF16, name=f"qtT{i}", tag=f"qtT{i}") for i in range(2)]
            vnp = [bpool.tile([128, D + 1], BF16, name=f"vnp{i}", tag=f"vnp{i}") for i in range(8)]
            for kt in range(8):
                nc.gpsimd.memset(vnp[kt][:, D:D + 1], 1.0)
            vmean_row = smpool.tile([1, D], BF16, tag="vmean_row")
            resid16 = bpool.tile([128, 2 * 512], BF16, name="resid16", tag="resid16")
            maxacc = smpool.tile([128, 8], F32, tag="maxacc")
            sumacc = smpool.tile([128, 8], F32, tag="sumacc")

            with tc.tile_pool(name="psA", bufs=2, space="PSUM") as psA:
                for wt, dst in ((wrb["k"], ktT), (wrb["q"], qtT)):
                    for et in range(2):
                        for nck in range(2):
                            ps = psA.tile([128, 512], F32, tag="psA")
                            for ft in range(2):
                                nc.tensor.matmul(
                                    ps[:],
                                    wt[ft][:, et * 128:(et + 1) * 128],
                                    xt[ft][:, nck * 512:(nck + 1) * 512],
                                    start=(ft == 0), stop=(ft == 1),
                                )
                            nc.scalar.copy(dst[et][:, nck * 512:(nck + 1) * 512], ps[:])

            # ---- phase C: QK + PE mask-add + fused max / sampled sum ----
            with tc.tile_pool(name="psQK", bufs=2, space="PSUM") as psQK:
                for qt in range(8):
                    am = mpoolA.tile([128, N], BF16, tag="am")
                    nc.sync.dma_start(am[:], am_d[qt * 128:(qt + 1) * 128, :])
                    ct2 = mpoolB.tile([128, N], BF16, tag="ct2")
                    nc.sync.dma_start(ct2[:], cnt_d[qt * 128:(qt + 1) * 128, :])
                    qk = psQK.tile([128, N], F32, tag="qk")
                    for kc in range(2):
                        for et in range(2):
                            nc.tensor.matmul(
                                qk[:, kc * 512:(kc + 1) * 512],
                                qtT[et][:, qt * 128:(qt + 1) * 128],
                                ktT[et][:, kc * 512:(kc + 1) * 512],
                                start=(et == 0), stop=False,
                            )
                        nc.tensor.matmul(
                            qk[:, kc * 512:(kc + 1) * 512], identb[:],
                            am[:, kc * 512:(kc + 1) * 512],
                            start=False, stop=True,
                        )
                    nc.vector.tensor_reduce(
                        maxacc[:, qt:qt + 1], qk[:], mybir.AxisListType.X, ALU.max
                    )
                    # (qk - 30000)*cnt == qk*cnt at sampled entries (cnt=0 off)
                    scrB = spoolB.tile([128, N], BF16, tag="scrB")
                    nc.vector.scalar_tensor_tensor(
                        scrB[:], qk[:], 1.0 / N, ct2[:], ALU.mult, ALU.mult,
                        accum_out=sumacc[:, qt:qt + 1],
                    )

            m_sb = smpool.tile([128, 8], F32, tag="m_sb")
            nc.vector.tensor_sub(m_sb[:], maxacc[:], sumacc[:])
            if stage == 1:
                nc.sync.dma_start(out_d[:, 0:8], m_sb[0:1, :])
                return

            # ---- phase D: PE-broadcast M, rank own half, selm, one-hots ----
            rank = smpool.tile([128, 4], F32, tag="rank")
            selm = smpool.tile([128, 4], F32, tag="selm")
            dsel = [smpool.tile([128, 512], BF16, name=f"dsel{i}", tag=f"dsel{i}")
                    for i in range(4)]
            with tc.tile_pool(name="psM", bufs=1, space="PSUM") as psM:
                psT = psM.tile([8, 128], F32, tag="psT")
                nc.tensor.transpose(psT[:], m_sb[:], ident32[:])
                m8 = smpool.tile([8, 128], F32, tag="m8")
                nc.scalar.copy(m8[:], psT[:])
                psm = psM.tile([128, N], F32, tag="psm")
                for r in range(8):
                    nc.tensor.matmul(
                        psm[:, r * 128:(r + 1) * 128],
                        sel8[:, r * 128:(r + 1) * 128], m8[:],
                        start=True, stop=True,
                    )
                for qt in range(4):
                    scr = (spoolA if qt < 2 else spoolB).tile([128, N], BF16, tag="scrR")
                    nc.vector.tensor_scalar(
                        scr[:], psm[:], m_sb[:, qt:qt + 1], None, ALU.is_gt,
                        ALU.add, accum_out=rank[:, qt:qt + 1],
                    )
            nc.vector.tensor_scalar(selm[:], rank[:], 139.5, None, ALU.is_le)
            for qt in range(4):
                nc.vector.tensor_scalar(
                    dsel[qt][:], nrow[:], qiota[:, qt:qt + 1], selm[:, qt:qt + 1],
                    ALU.is_equal, ALU.mult,
                )
            if stage == 2:
                nc.sync.dma_start(out_d[:, 0:4], rank[0:1, :])
                nc.sync.dma_start(out_d[:, 4:8], selm[0:1, :])
                return

            # ---- phase B2 (PE fills rank latency): V, vmean, residual ----
            with tc.tile_pool(name="psB", bufs=2, space="PSUM") as psB:
                for kt in range(8):
                    ps = psB.tile([128, D], F32, tag="psb2")
                    for ft in range(2):
                        nc.tensor.matmul(
                            ps[:], xt[ft][:, kt * 128:(kt + 1) * 128],
                            wrb["v"][ft][:],
                            start=(ft == 0), stop=(ft == 1),
                        )
                    nc.scalar.copy(vnp[kt][:, 0:D], ps[:])
                psvm = psB.tile([1, D], F32, tag="psvm", bufs=1)
                for kt in range(8):
                    nc.tensor.matmul(
                        psvm[:], ones16[:], vnp[kt][:, 0:D],
                        start=(kt == 0), stop=(kt == 7),
                    )
                nc.scalar.mul(vmean_row[:], psvm[:], 1.0 / N)
                for dtl in range(2):
                    ps = psB.tile([128, 512], F32, tag="psrd", bufs=2)
                    for ft in range(2):
                        nc.tensor.matmul(
                            ps[:], wrb["a"][ft][:, dtl * 128:(dtl + 1) * 128],
                            xt[ft][:, 0:512],
                            start=(ft == 0), stop=(ft == 1),
                        )
                    nc.scalar.activation(
                        resid16[:, dtl * 512:(dtl + 1) * 512], ps[:],
                        ACTF.Identity, bias=badd[dtl][:], scale=1.0,
                    )

            # ---- phase E: scores^T for all own-half queries, exp, attn@V ----
            expdT = [bpool.tile([128, 512], BF16, name=f"expdT{i}", tag=f"expdT{i}")
                     for i in range(8)]
            aug = [smpool.tile([128, D], BF16, name=f"aug{i}", tag=f"aug{i}")
                   for i in range(4)]
            ctxh = bpool.tile([128, 2 * 512], BF16, name="ctxh", tag="ctxh")
            facc = smpool.tile([128, 16], F32, tag="facc")
            nc.gpsimd.memset(facc[:, NCLS:16], 0.0)
            with tc.tile_pool(name="psC", bufs=2, space="PSUM") as psC, \
                 tc.tile_pool(name="psE", bufs=2, space="PSUM") as psE:
                for kt in range(8):
                    ps = psC.tile([128, 512], F32, tag="psC")
                    for et in range(2):
                        nc.tensor.matmul(
                            ps[:], ktT[et][:, kt * 128:(kt + 1) * 128],
                            qtT[et][:, 0:512],
                            start=(et == 0), stop=(et == 1),
                        )
                    nc.scalar.activation(
                        expdT[kt][:], ps[:], ACTF.Exp, scale=1.0 / math.sqrt(D)
                    )
                for qc in range(4):
                    pse = psE.tile([128, D + 1], F32, tag="pse")
                    for kt in range(8):
                        nc.tensor.matmul(
                            pse[:], expdT[kt][:, qc * 128:(qc + 1) * 128], vnp[kt][:],
                            start=(kt == 0), stop=(kt == 7),
                        )
                    rc = smpool.tile([128, 1], F32, tag=f"rc{qc}")
                    nc.vector.reciprocal(rc[:], pse[:, D:D + 1])
                    nc.vector.tensor_scalar(
                        aug[qc][:], pse[:, 0:D], rc[:], None, ALU.mult
                    )
            if stage == 4:
                nc.sync.dma_start(out_d[:, :], aug[0][0:1, 0:16].bitcast(BF16))
                return

            # ---- phase F: scatter + fill + residual add + fused dots ----
            with tc.tile_pool(name="psF", bufs=2, space="PSUM") as psF, \
                 tc.tile_pool(name="psCt", bufs=1, space="PSUM") as psCt:
                cnt_ps = psCt.tile([1, 512], F32, tag="cnt_ps")
                for qc in range(4):
                    nc.tensor.matmul(cnt_ps[:], ones16[:], dsel[qc][:],
                                     start=(qc == 0), stop=(qc == 3))
                fill16 = smpool.tile([1, 512], BF16, tag="fill16")
                nc.scalar.activation(fill16[:], cnt_ps[:], ACTF.Copy,
                                     bias=1.0, scale=-1.0)
                for dtl in range(2):
                    ps = psF.tile([128, 512], F32, tag="psF")
                    for qc in range(4):
                        nc.tensor.matmul(
                            ps[:], aug[qc][:, dtl * 128:(dtl + 1) * 128], dsel[qc][:],
                            start=(qc == 0), stop=False,
                        )
                    nc.tensor.matmul(
                        ps[:], vmean_row[0:1, dtl * 128:(dtl + 1) * 128], fill16[:],
                        start=False, stop=True,
                    )
                    nc.vector.scalar_tensor_tensor(
                        ctxh[:, dtl * 512:(dtl + 1) * 512], ps[:], 1.0,
                        resid16[:, dtl * 512:(dtl + 1) * 512], ALU.mult, ALU.add,
                    )
                if stage == 5:
                    nc.sync.dma_start(out_d[:, :], ctxh[0:1, 0:16].bitcast(BF16))
                    return
                for cls in range(NCLS):
                    scr = (spoolA if cls % 2 else spoolB).tile([128, N], BF16, tag="scrD")
                    nc.vector.scalar_tensor_tensor(
                        scr[:], ctxh[:], 1.0, wf[cls][:], ALU.mult, ALU.mult,
                        accum_out=facc[:, cls:cls + 1],
                    )
            with tc.tile_pool(name="psO", bufs=1, space="PSUM") as psO:
                o = psO.tile([1, 16], F32, tag="o")
                nc.tensor.matmul(o[:], onesr32[:], facc[:], start=True, stop=True)
                osb = smpool.tile([1, 16], F32, tag="osb")
                nc.scalar.copy(osb[:], o[:])
                nc.sync.dma_start(out_d[:, :], osb[:])

    with tile.TileContext(nc) as tc:
        emit(tc)
    nc.compile()
    return nc


_NC_CACHE = {}


def get_nc(stage=9):
    if stage not in _NC_CACHE:
        _NC_CACHE[stage] = build_nc(stage)
    return _NC_CACHE[stage]


def host_prep(inputs):
    """Build per-core input maps from the full problem inputs (layout only)."""
    x = np.asarray(inputs["input_embedding"], np.float32)        # [B, N, D]
    wq = np.asarray(inputs["Wq"], np.float32)
    wk = np.asarray(inputs["Wk"], np.float32)
    wv = np.asarray(inputs["Wv"], np.float32)
    wa = np.asarray(inputs["Wadd"], np.float32)
    badd = np.asarray(inputs["badd"], np.float32)
    wfin = np.asarray(inputs["Wfin"], np.float32)                # [10, N*D]
    idx = np.asarray(inputs["index_sample"]).astype(np.int64)    # [N, U]
    bf = ml_dtypes.bfloat16

    cnt = np.zeros((N, N), np.float32)
    np.add.at(cnt, (np.arange(N)[:, None], idx), 1.0)

    # Core half h=1 gets the n-axis halves swapped on every n-indexed input
    # (the pipeline is equivariant under a joint permutation of X rows,
    # mask rows+cols, and Wfin columns), so "columns 0:512" is its half.
    perms = [np.arange(N), np.concatenate([np.arange(512, N), np.arange(512)])]
    am_h, cnt_h = [], []
    for p in perms:
        cp = cnt[p][:, p]
        am_h.append(np.where(cp > 0, 0.0, NEG).astype(bf))
        cnt_h.append(cp.astype(bf))

    # Wfin[c, n*256+d] -> [10, d, n_local] -> [10, 128, 2*512] bf16
    wr = wfin.reshape(NCLS, N, D).transpose(0, 2, 1)             # [10, 256, 1024]
    wr_h = [
        np.ascontiguousarray(
            wr[:, :, perms[h][:512]].reshape(NCLS, 2, 128, 512)
            .transpose(0, 2, 1, 3)
        ).reshape(NCLS, 128, N).astype(bf)
        for h in range(2)
    ]

    w_all = np.stack([w.T.reshape(2, 128, D) for w in (wq, wk, wv, wa)])
    misc = np.zeros((128, 518), np.float32)
    misc[:, 0:512] = np.arange(512, dtype=np.float32)[None, :]
    misc[:, 512:516] = (np.arange(128, dtype=np.float32)[:, None]
                        + 128.0 * np.arange(4, dtype=np.float32)[None, :])
    misc[:, 516] = badd[0:128]
    misc[:, 517] = badd[128:256]
    sel8 = np.zeros((8, 1024), np.float32)
    for r in range(8):
        sel8[r, r * 128:(r + 1) * 128] = 1.0
    consts = {
        "w_all_h": np.ascontiguousarray(w_all.reshape(8, 128, D)).astype(bf),
        "misc": misc,
        "ident32": np.eye(128, dtype=np.float32),
        "identb": np.eye(128, dtype=np.float32).astype(bf),
        "sel8": sel8,
    }

    in_maps = []
    xt_cache = {}
    for c in range(8):
        b, h = c // 2, c % 2
        m = dict(consts)
        if (b, h) not in xt_cache:
            xp = np.ascontiguousarray(x[b][perms[h]])
            xt_cache[(b, h)] = np.ascontiguousarray(xp.T).astype(bf)
        m["xt_h"] = xt_cache[(b, h)]
        m["am_b"] = am_h[h]
        m["cnt_b"] = cnt_h[h]
        m["wfin_h"] = wr_h[h]
        in_maps.append(m)
    return in_maps


def host_combine(results, inputs):
    bfin = np.asarray(inputs["bfin"], np.float32)
    out = np.zeros((B, NCLS), np.float32)
    for c in range(8):
        b = c // 2
        out[b] += results[c]["out10"].reshape(-1)[0:NCLS]
    return out + bfin[None, :]


def kernel(**inputs):
    nc = get_nc()
    in_maps = host_prep(inputs)
    res = run_bass_kernel_spmd(nc, in_maps, core_ids=list(range(8)))
    return host_combine(res.results, inputs)
